# revision 17
# baseline (speedup 1.0000x reference)
"""Trainium2 Bass kernel for nn_AsymmetricLossCustomPrioritySmallFocal.

Data-parallel over batch across 8 NeuronCores. Each core processes 256 rows
(2 blocks of 128 partition-rows x 9728 padded cols).

Per element (y in {0,1}):
  w  = sigmoid(x); L1 = ln(w); L2 = ln(1.05 - w); r = relu(w - 0.05)
  B  = L2 * r^4            (y=0 contribution: log(xs_neg)*(1-xs_neg)^4)
  A  = L1 * (1 - w)        (y=1 contribution: log(xs_pos)*(1-xs_pos))
  sum contrib0 = sum(B) + sum(y*A) - sum(y*B)
Top-10-per-row correction for whitelist-priority upweighting:
  corr_elem = cond * (factor-1) * contrib0   (only 10 elements per row)

Optimizations:
- bf16 intermediates (DVE 2x mode); y shipped as bf16 (0/1, lossless).
- Columns host-permuted so all cat!=4 classes land in cols [0, SP): the
  per-row whitelist presence sums reduce over a [128, SP] slice instead of
  the full row.
- Scalar reductions via ones-matvec on the idle PE (PSUM accumulation).
- Top-k: per-segment vector.max (16 segs) -> 128 candidates -> top-10 of
  candidates; global indices recovered with two full-row max_index scans.
Output = -(sum contrib0 + corr). Host sums the 8 per-core [128,8] partials.
"""
import os
from contextlib import ExitStack
import numpy as np
import ml_dtypes

import concourse.bass as bass
import concourse.bacc as bacc
import concourse.tile as tile
from concourse import mybir
from concourse.bass_utils import run_bass_kernel_spmd

F32 = mybir.dt.float32
BF16 = mybir.dt.bfloat16
U32 = mybir.dt.uint32
I32 = mybir.dt.int32
U8 = mybir.dt.uint8
ALU = mybir.AluOpType
ACT = mybir.ActivationFunctionType
AXX = mybir.AxisListType.X

B_GLOBAL, C_GLOBAL = 2048, 9605
NCORES = 8
P = 128
CP = 9728
RPC = B_GLOBAL // NCORES          # 256 rows per core
NBLK = RPC // P                   # 2
FT = 1216                         # elementwise column tile
NT = CP // FT                     # 8
SP = 192                          # special-column region (cat != 4), padded
NSEG = 16                         # top-k segments per row
SEGW = CP // NSEG                 # 608
NEG_BIG = -1e30

N_CORES_RUN = int(os.environ.get("K_NCORES", "8"))
NREP = int(os.environ.get("K_NREP", "1"))

_COMPILED = {}


def _register_const(nc, val, dtype=F32):
    t = nc.alloc_sbuf_tensor(f"const-{dtype.name}-{val}", [128, 1], dtype)
    nc.gpsimd.memset(t.ap(), val)
    nc.const_aps.aps[(dtype, val)] = t.ap()


def _build():
    nc = bacc.Bacc("TRN2", target_bir_lowering=False, debug=False)
    _register_const(nc, 1.05)
    _register_const(nc, -0.05)
    nc.all_engine_barrier()
    x_d = nc.declare_dram_parameter("x", [RPC, CP], F32, isOutput=False)
    y_d = nc.declare_dram_parameter("y", [RPC, CP], BF16, isOutput=False)
    mv_d = nc.declare_dram_parameter("mvec", [P, SP], BF16, isOutput=False)
    av_d = nc.declare_dram_parameter("avec", [CP], F32, isOutput=False)
    out_d = nc.declare_dram_parameter("out", [P, 8], F32, isOutput=True)

    with tile.TileContext(nc) as tc:
        _body(tc, nc, x_d, y_d, mv_d, av_d, out_d)
    nc.finalize()
    return nc


def _mm_reduce(nc, ps, ones, src, started):
    """Accumulate sum over (partitions x free) of src into psum row ps[1,512]."""
    n = src.shape[-1]
    for c0 in range(0, n, 512):
        c1 = min(c0 + 512, n)
        nc.tensor.matmul(out=ps[:, 0:(c1 - c0)], lhsT=ones[:],
                         rhs=src[:, c0:c1], start=not started, stop=False,
                         skip_group_check=True)
        started = True
    return started


def _body(tc, nc, x_d, y_d, mv_d, av_d, out_d):
    ctx = ExitStack()
    xp = ctx.enter_context(tc.tile_pool(name="xp", bufs=2))
    yp = ctx.enter_context(tc.tile_pool(name="yp", bufs=3))
    wp = ctx.enter_context(tc.tile_pool(name="wp", bufs=9))
    l1p = ctx.enter_context(tc.tile_pool(name="l1p", bufs=3))
    l2p = ctx.enter_context(tc.tile_pool(name="l2p", bufs=3))
    rp = ctx.enter_context(tc.tile_pool(name="rp", bufs=3))
    bp = ctx.enter_context(tc.tile_pool(name="bp", bufs=3))
    zp = ctx.enter_context(tc.tile_pool(name="zp", bufs=3))
    wnp = ctx.enter_context(tc.tile_pool(name="wnp", bufs=3))
    mvp = ctx.enter_context(tc.tile_pool(name="mvp", bufs=1))
    accp = ctx.enter_context(tc.tile_pool(name="accp", bufs=40))
    tkp = ctx.enter_context(tc.tile_pool(name="tkp", bufs=2))
    psp = ctx.enter_context(tc.tile_pool(name="psp", bufs=1, space="PSUM"))

    ones = mvp.tile([P, 1], BF16, tag="ones")
    nc.vector.memset(ones[:], 1.0)

    mv = mvp.tile([P, SP], BF16, tag="mv")
    nc.sync.dma_start(out=mv[:], in_=mv_d.ap())

    psB = psp.tile([1, 512], F32, tag="psB")
    psYB = psp.tile([1, 512], F32, tag="psYB")
    psA = psp.tile([1, 512], F32, tag="psA")
    stB = stYB = stA = False

    corr_accs = []
    for rep in range(NREP):
      for b in range(NBLK):
        rows = slice(b * P, (b + 1) * P)
        xb = xp.tile([P, CP], F32, tag="xb")
        for t in range(NT):
            sl = slice(t * FT, (t + 1) * FT)
            nc.sync.dma_start(out=xb[:, sl], in_=x_d.ap()[rows, sl])

        # function-major ACT order: all sigmoids first, then Ln/Square
        # passes (Square shares every activation table set) -> 2 table
        # loads per block instead of 2 per tile.
        ws = []
        for t in range(NT):
            sl = slice(t * FT, (t + 1) * FT)
            w = wp.tile([P, FT], F32, tag="w")
            nc.scalar.activation(w[:], xb[:, sl], ACT.Sigmoid)
            ws.append(w)
        accM = None
        for t in range(NT):
            sl = slice(t * FT, (t + 1) * FT)
            w = ws[t]
            yt = yp.tile([P, FT], BF16, tag="yt")
            nc.sync.dma_start(out=yt[:], in_=y_d.ap()[rows, sl])

            l1 = l1p.tile([P, FT], BF16, tag="l1")
            nc.scalar.activation(l1[:], w[:], ACT.Ln)
            l2 = l2p.tile([P, FT], BF16, tag="l2")
            nc.scalar.activation(l2[:], w[:], ACT.Ln, bias=1.05, scale=-1.0)

            # r^4 ~= ((w-0.05)^2)^2 on ACT+DVE; relu dropped: for w<0.05 the
            # spurious contribution is bounded by ln(1.05)*0.05^4 ~ 3e-7/elem.
            r = rp.tile([P, FT], BF16, tag="r")
            nc.scalar.activation(r[:], w[:], ACT.Square, bias=-0.05)
            nc.vector.tensor_tensor(out=r[:], in0=r[:], in1=r[:], op=ALU.mult)
            bt = bp.tile([P, FT], BF16, tag="bt")
            nc.vector.tensor_tensor(out=bt[:], in0=l2[:], in1=r[:], op=ALU.mult)
            stB = _mm_reduce(nc, psB, ones, bt, stB)

            # p-bits: only the special-column slice matters (tile 0)
            if t == 0:
                ymt = rp.tile([P, SP], BF16, tag="ymt")
                nc.vector.tensor_tensor(out=ymt[:], in0=yt[:, 0:SP],
                                        in1=mv[:], op=ALU.mult)
                ymr = accp.tile([P, 1], F32, tag="acc")
                nc.vector.tensor_reduce(ymr[:], ymt[:], AXX, ALU.add)
                accM = ymr

            # Z = y*L1; wn = 1-w; Zw = Z*wn in place
            z = zp.tile([P, FT], BF16, tag="z")
            nc.vector.tensor_tensor(out=z[:], in0=yt[:], in1=l1[:], op=ALU.mult)
            wn = wnp.tile([P, FT], BF16, tag="wn")
            nc.vector.tensor_scalar(wn[:], w[:], 1.0, -1.0, ALU.subtract, ALU.mult)
            nc.vector.tensor_tensor(out=z[:], in0=z[:], in1=wn[:], op=ALU.mult)
            stA = _mm_reduce(nc, psA, ones, z, stA)

            # yB = y*B (overwrite yt; dead after)
            nc.vector.tensor_tensor(out=yt[:], in0=yt[:], in1=bt[:], op=ALU.mult)
            stYB = _mm_reduce(nc, psYB, ones, yt, stYB)

        # ---------- top-k with self-indexing values ----------
        # Stamp the column index (< 2^14) into the low 16 mantissa bits of x
        # in place (after all elementwise consumers have read xb). Perturbs x
        # by <= 2^-8 relative, harmless for top-k selection and the tiny
        # correction, and makes every max value carry its own column index.
        xb16 = xb[:].bitcast(mybir.dt.uint16)
        nc.gpsimd.iota(xb16[:, 0:2 * CP:2], pattern=[[1, CP]], base=0,
                       channel_multiplier=0)
        cands = tkp.tile([P, NSEG * 8], F32, tag="cands")
        for s in range(NSEG):
            nc.vector.max(out=cands[:, s * 8:(s + 1) * 8],
                          in_=xb[:, s * SEGW:(s + 1) * SEGW])
        top8 = tkp.tile([P, 8], F32, tag="top8")
        nc.vector.max(out=top8[:], in_=cands[:])
        nc.vector.match_replace(out=cands[:], in_to_replace=top8[:],
                                in_values=cands[:], imm_value=NEG_BIG)
        nxt8 = tkp.tile([P, 8], F32, tag="nxt8")
        nc.vector.max(out=nxt8[:], in_=cands[:])

        tv = tkp.tile([P, 10], F32, tag="tv")
        nc.vector.tensor_copy(tv[:, 0:8], top8[:])
        nc.vector.tensor_copy(tv[:, 8:10], nxt8[:, 0:2])
        ti = tkp.tile([P, 10], U32, tag="ti")
        nc.vector.tensor_copy(ti[:], tv[:].bitcast(mybir.dt.uint16)[:, 0:20:2])

        # ---------- gathers ----------
        y_at = tkp.tile([P, 10], BF16, tag="y_at")
        a_at = tkp.tile([P, 10], F32, tag="a_at")
        rb = tkp.tile([P, 1], I32, tag="rb")
        nc.gpsimd.iota(rb[:], pattern=[[0, 1]], base=b * P * CP,
                       channel_multiplier=CP)
        rbf = tkp.tile([P, 1], F32, tag="rbf")
        nc.vector.tensor_copy(rbf[:], rb[:])
        tif = tkp.tile([P, 10], F32, tag="tif")
        nc.vector.tensor_copy(tif[:], ti[:])
        nc.vector.tensor_scalar(tif[:], tif[:], rbf[:, 0:1], None, ALU.add)
        flat = tkp.tile([P, 10], U32, tag="flat")
        nc.vector.tensor_copy(flat[:], tif[:])
        for j in range(10):
            nc.gpsimd.indirect_dma_start(
                out=y_at[:, j:j + 1], out_offset=None,
                in_=y_d.ap().flatten().unsqueeze(1),
                in_offset=bass.IndirectOffsetOnAxis(ap=flat[:, j:j + 1], axis=0))
            nc.gpsimd.indirect_dma_start(
                out=a_at[:, j:j + 1], out_offset=None,
                in_=av_d.ap().unsqueeze(1),
                in_offset=bass.IndirectOffsetOnAxis(ap=ti[:, j:j + 1], axis=0))

        # ---------- p-bit decode ----------
        Sm = accM
        p3 = tkp.tile([P, 1], F32, tag="p3")
        nc.vector.tensor_scalar(p3[:], Sm[:], 16384.0, None, ALU.is_ge)
        t3i = tkp.tile([P, 1], I32, tag="t3i")
        nc.vector.tensor_scalar(t3i[:], Sm[:], 1.0 / 16384.0, None, ALU.mult)
        t3f = tkp.tile([P, 1], F32, tag="t3f")
        nc.vector.tensor_copy(t3f[:], t3i[:])
        u1 = tkp.tile([P, 1], F32, tag="u1")
        nc.vector.tensor_scalar(u1[:], t3f[:], -16384.0, None, ALU.mult)
        S2 = tkp.tile([P, 1], F32, tag="S2")
        nc.vector.tensor_tensor(out=S2[:], in0=Sm[:], in1=u1[:], op=ALU.add)
        p2 = tkp.tile([P, 1], F32, tag="p2")
        nc.vector.tensor_scalar(p2[:], S2[:], 128.0, None, ALU.is_ge)
        t2i = tkp.tile([P, 1], I32, tag="t2i")
        nc.vector.tensor_scalar(t2i[:], S2[:], 1.0 / 128.0, None, ALU.mult)
        t2f = tkp.tile([P, 1], F32, tag="t2f")
        nc.vector.tensor_copy(t2f[:], t2i[:])
        u2 = tkp.tile([P, 1], F32, tag="u2")
        nc.vector.tensor_scalar(u2[:], t2f[:], -128.0, None, ALU.mult)
        S1 = tkp.tile([P, 1], F32, tag="S1")
        nc.vector.tensor_tensor(out=S1[:], in0=S2[:], in1=u2[:], op=ALU.add)
        p1 = tkp.tile([P, 1], F32, tag="p1")
        nc.vector.tensor_scalar(p1[:], S1[:], 0.5, None, ALU.is_ge)
        h = tkp.tile([P, 1], F32, tag="h")
        nc.vector.tensor_tensor(out=h[:], in0=p1[:], in1=p2[:], op=ALU.max)
        nc.vector.tensor_tensor(out=h[:], in0=h[:], in1=p3[:], op=ALU.max)
        h4 = tkp.tile([P, 1], F32, tag="h4")
        nc.vector.tensor_scalar(h4[:], h[:], 1.0, -1.0, ALU.subtract, ALU.mult)

        # ---------- tiny correction on [P, 10] ----------
        wt = tkp.tile([P, 10], F32, tag="wt")
        nc.scalar.activation(wt[:], tv[:], ACT.Sigmoid)
        l1t = tkp.tile([P, 10], F32, tag="l1t")
        nc.scalar.activation(l1t[:], wt[:], ACT.Ln)
        l2t = tkp.tile([P, 10], F32, tag="l2t")
        nc.scalar.activation(l2t[:], wt[:], ACT.Ln, bias=1.05, scale=-1.0)

        rt = tkp.tile([P, 10], F32, tag="rt")
        nc.vector.tensor_scalar(rt[:], wt[:], 0.05, 0.0, ALU.subtract, ALU.max)
        nc.vector.tensor_tensor(out=rt[:], in0=rt[:], in1=rt[:], op=ALU.mult)
        nc.vector.tensor_tensor(out=rt[:], in0=rt[:], in1=rt[:], op=ALU.mult)
        btt = tkp.tile([P, 10], F32, tag="btt")
        nc.vector.tensor_tensor(out=btt[:], in0=l2t[:], in1=rt[:], op=ALU.mult)
        wnt = tkp.tile([P, 10], F32, tag="wnt")
        nc.vector.tensor_scalar(wnt[:], wt[:], 1.0, -1.0, ALU.subtract, ALU.mult)
        att = tkp.tile([P, 10], F32, tag="att")
        nc.vector.tensor_tensor(out=att[:], in0=l1t[:], in1=wnt[:], op=ALU.mult)
        xnt = tkp.tile([P, 10], F32, tag="xnt")
        nc.vector.tensor_scalar(xnt[:], wt[:], 1.05, -1.0, ALU.subtract, ALU.mult)
        nc.vector.tensor_scalar(xnt[:], xnt[:], 1.0, None, ALU.min)
        fm1 = tkp.tile([P, 10], F32, tag="fm1")
        nc.vector.tensor_scalar(fm1[:], xnt[:], 2.0, 1.0, ALU.mult, ALU.subtract)
        fm0 = tkp.tile([P, 10], F32, tag="fm0")
        nc.vector.tensor_scalar(fm0[:], wt[:], 2.0, 1.0, ALU.mult, ALU.subtract)

        ymk = tkp.tile([P, 10], U8, tag="ymk")
        nc.vector.tensor_scalar(ymk[:], y_at[:], 0.5, None, ALU.is_ge)
        ct = tkp.tile([P, 10], F32, tag="ct")
        nc.vector.tensor_copy(ct[:], btt[:])
        nc.vector.copy_predicated(ct[:], ymk[:], att[:])
        ftl = tkp.tile([P, 10], F32, tag="ftl")
        nc.vector.tensor_copy(ftl[:], fm0[:])
        nc.vector.copy_predicated(ftl[:], ymk[:], fm1[:])

        im = tkp.tile([P, 10], F32, tag="im")
        nc.vector.tensor_scalar(im[:], a_at[:], 7.5, None, ALU.is_ge)
        im8 = tkp.tile([P, 10], F32, tag="im8")
        nc.vector.tensor_scalar(im8[:], im[:], 8.0, None, ALU.mult)
        catv = tkp.tile([P, 10], F32, tag="catv")
        nc.vector.tensor_tensor(out=catv[:], in0=a_at[:], in1=im8[:], op=ALU.subtract)

        condB = tkp.tile([P, 10], F32, tag="condB")
        cx = tkp.tile([P, 10], F32, tag="cx")
        first = True
        for val, pf in [(1.0, p1), (2.0, p2), (3.0, p3), (4.0, h4)]:
            nc.vector.tensor_scalar(cx[:], catv[:], val, None, ALU.is_equal)
            nc.vector.tensor_tensor(out=cx[:], in0=cx[:],
                                    in1=pf[:, 0:1].to_broadcast([P, 10]), op=ALU.mult)
            if first:
                nc.vector.tensor_copy(condB[:], cx[:])
                first = False
            else:
                nc.vector.tensor_tensor(out=condB[:], in0=condB[:], in1=cx[:],
                                        op=ALU.add)

        nc.vector.tensor_scalar(im[:], im[:], 1.0, -1.0, ALU.subtract, ALU.mult)
        nc.vector.tensor_tensor(out=im[:], in0=im[:],
                                in1=h4[:, 0:1].to_broadcast([P, 10]), op=ALU.mult)
        cond = tkp.tile([P, 10], F32, tag="cond")
        nc.vector.tensor_tensor(out=cond[:], in0=im[:], in1=condB[:], op=ALU.max)
        nc.vector.tensor_tensor(out=cond[:], in0=cond[:], in1=ftl[:], op=ALU.mult)
        nc.vector.tensor_tensor(out=cond[:], in0=cond[:], in1=ct[:], op=ALU.mult)
        cb = accp.tile([P, 1], F32, tag="acc")
        nc.vector.tensor_reduce(cb[:], cond[:], AXX, ALU.add)
        corr_accs.append(cb)

    # ---------- output ----------
    sb = tkp.tile([1, 512], F32, tag="sb")
    ot = tkp.tile([P, 8], F32, tag="ot")
    nc.vector.memset(ot[:], 0.0)
    for k, ps in [(0, psB), (1, psYB), (2, psA)]:
        nc.vector.tensor_copy(sb[:], ps[:])
        nc.vector.tensor_reduce(ot[0:1, k:k + 1], sb[:], AXX, ALU.add)
    c01 = accp.tile([P, 1], F32, tag="acc")
    nc.vector.tensor_tensor(out=c01[:], in0=corr_accs[-2][:], in1=corr_accs[-1][:],
                            op=ALU.add)
    nc.vector.tensor_copy(ot[:, 3:4], c01[:])
    nc.sync.dma_start(out=out_d.ap(), in_=ot[:])
    ctx.close()


def _prep_inputs(x, y, cat, in_mapping):
    """Host-side shard prep: column permutation (cat!=4 first), padding,
    lossless bf16 repack of y, per-core split, tiny metadata vectors."""
    x = np.asarray(x, dtype=np.float32)
    y = np.asarray(y, dtype=np.float32)
    cat = np.asarray(cat)
    in_mapping = np.asarray(in_mapping)

    special = np.where(cat != 4)[0]
    normal = np.where(cat == 4)[0]
    assert len(special) <= SP, f"too many special columns: {len(special)}"
    perm = np.concatenate([special, normal])
    catp = cat[perm]
    imp = in_mapping[perm]

    xp_ = np.full((B_GLOBAL, CP), -4.0, np.float32)
    xp_[:, :C_GLOBAL] = x[:, perm]
    yp_ = np.zeros((B_GLOBAL, CP), np.float32)
    yp_[:, :C_GLOBAL] = y[:, perm]
    yp_b = yp_.astype(ml_dtypes.bfloat16)

    ns = len(special)
    mvec = np.zeros(SP, np.float32)
    mvec[:ns] = ((catp[:ns] == 1) * 1.0 + (catp[:ns] == 2) * 128.0
                 + (catp[:ns] == 3) * 16384.0)
    mvec_rep = np.ascontiguousarray(
        np.broadcast_to(mvec, (P, SP))).astype(ml_dtypes.bfloat16)
    avec = np.full(CP, 4.0, np.float32)
    avec[:C_GLOBAL] = catp.astype(np.float32) + 8.0 * imp.astype(np.float32)

    in_maps = []
    for c in range(NCORES):
        rows = slice(c * RPC, (c + 1) * RPC)
        in_maps.append({
            "x": np.ascontiguousarray(xp_[rows]),
            "y": np.ascontiguousarray(yp_b[rows]),
            "mvec": mvec_rep,
            "avec": avec,
        })
    return in_maps


def kernel(x, y, cat, in_mapping, _want_trace=False):
    if "nc" not in _COMPILED:
        _COMPILED["nc"] = _build()
    nc = _COMPILED["nc"]
    in_maps = _prep_inputs(x, y, cat, in_mapping)
    res = run_bass_kernel_spmd(nc, in_maps[:N_CORES_RUN],
                               core_ids=list(range(N_CORES_RUN)),
                               trace=_want_trace)
    total = 0.0
    for core_out in res.results:
        o = core_out["out"].astype(np.float64)
        total += (o[:, 0].sum() - o[:, 1].sum() + o[:, 2].sum()
                  + o[:, 3].sum())
    ans = np.float32(-total)
    if _want_trace:
        return ans, res
    return ans
